# revision 1
# baseline (speedup 1.0000x reference)
"""Bass/Trainium2 kernel for nn_BayesMultiheadAttention (B=4,T=2048,D=1024,H=8).

Sharding: tensor-parallel over heads. Core c computes head c (QKV proj +
causal attention) for all 4 batches; an AllToAll redistributes per-head
outputs into per-token-slice outputs; each core then does the
multiplicative reduce over heads and its 1/8 token slice of out_proj.

All matmuls run in float32r (fp32 rounded to 11 mantissa bits, full PE
rate) except the softmax-denominator ones-matmul which is exact fp32.
Dropout masks and the 1/sqrt(HD) scale are folded into per-(core,batch)
weight copies on the host. Softmax skips max-subtraction (scores are
O(5), exp cannot overflow) so it needs only exp, a partition-sum and one
reciprocal.
"""
import numpy as np

import concourse.bacc as bacc
import concourse.mybir as mybir
import concourse.tile as tile
from concourse.bass_utils import run_bass_kernel_spmd

B, T, D, H = 4, 2048, 1024, 8
HD = 128          # head dim
P = 128           # partitions
NC = 8            # cores
TQ = 512          # qt chunk width
NKD = D // P      # 8 contraction tiles
NTT = T // P      # 16 token tiles per batch
NQC = T // TQ     # 4 qt chunks per batch
TOK_SLICE = B * T // NC  # 1024 tokens per core in the tail

dt = mybir.dt
F32 = dt.float32
F32R = dt.float32r

_PROGRAM = None


def build_program():
    global _PROGRAM
    if _PROGRAM is not None:
        return _PROGRAM
    nc = bacc.Bacc("TRN2", target_bir_lowering=False, debug=False,
                   num_devices=NC)

    xT_d = nc.dram_tensor("xT", [B, D, T], F32, kind="ExternalInput")
    wq_d = nc.dram_tensor("wq", [B, NKD, P, HD], F32, kind="ExternalInput")
    wk_d = nc.dram_tensor("wk", [B, NKD, P, HD], F32, kind="ExternalInput")
    wv_d = nc.dram_tensor("wv", [B, NKD, P, HD], F32, kind="ExternalInput")
    wo_d = nc.dram_tensor("wo", [HD, D], F32, kind="ExternalInput")
    cm_d = nc.dram_tensor("cm", [4, P, TQ], F32, kind="ExternalInput")
    y_d = nc.dram_tensor("y", [TOK_SLICE, D], F32, kind="ExternalOutput")

    rg = [list(range(NC))]
    Exp = mybir.ActivationFunctionType.Exp

    with tile.TileContext(nc) as tc:
        with (
            tc.tile_pool(name="const", bufs=1) as constp,
            tc.tile_pool(name="xr", bufs=1) as xrp,
            tc.tile_pool(name="xs", bufs=2) as xsp,
            tc.tile_pool(name="wst", bufs=2) as wstp,
            tc.tile_pool(name="wr", bufs=1) as wrp,
            tc.tile_pool(name="qkv", bufs=1) as qkvp,
            tc.tile_pool(name="eo", bufs=3) as eop,
            tc.tile_pool(name="sc", bufs=2) as scp,
            tc.tile_pool(name="outb", bufs=1) as outbp,
            tc.tile_pool(name="tail", bufs=1) as tailp,
            tc.tile_pool(name="hp", bufs=3) as hpp,
            tc.tile_pool(name="ysb", bufs=2) as ysbp,
            tc.tile_pool(name="psA", bufs=2, space="PSUM") as psA,
            tc.tile_pool(name="psS", bufs=2, space="PSUM") as psS,
            tc.tile_pool(name="psO", bufs=2, space="PSUM") as psO,
            tc.tile_pool(name="dram", bufs=1, space="DRAM") as dram,
        ):
            a2a_in = dram.tile([NC, P, TOK_SLICE], F32)
            a2a_out = dram.tile([NC, P, TOK_SLICE], F32)

            ones128 = constp.tile([P, P], F32, name="ones128", tag="ones128")
            nc.vector.memset(ones128[:], 1.0)

            cm_sb = constp.tile([P, 4 * TQ], F32, name="cm_sb", tag="cm_sb")
            nc.sync.dma_start(cm_sb[:], cm_d.ap().rearrange("j p q -> p j q"))

            wo_st = wstp.tile([P, D], F32, name="wo_st", tag="wst")
            nc.sync.dma_start(wo_st[:], wo_d.ap())
            wor = constp.tile([P, D], F32R, name="wor", tag="wor")
            nc.vector.tensor_copy(wor[:], wo_st[:])

            for b in range(B):
                # ---- stage + round x^T (d on partitions, tok free) ----
                xr = xrp.tile([P, NKD * T], F32R, name="xr", tag="xr")
                for kd in range(NKD):
                    xs = xsp.tile([P, T], F32, name="xs", tag="xs")
                    nc.sync.dma_start(
                        xs[:], xT_d.ap()[b, kd * P:(kd + 1) * P, :])
                    nc.vector.tensor_copy(xr[:, kd * T:(kd + 1) * T], xs[:])

                # ---- stage + round weights for this (core, batch) ----
                wr = {}
                for nm, wd in (("q", wq_d), ("k", wk_d), ("v", wv_d)):
                    ws = wstp.tile([P, NKD * HD], F32, name=f"ws_{nm}",
                                   tag="wst")
                    nc.sync.dma_start(
                        ws[:], wd.ap()[b].rearrange("kd p m -> p kd m"))
                    wt = wrp.tile([P, NKD * HD], F32R, name=f"wr_{nm}",
                                  tag=f"wr_{nm}")
                    nc.vector.tensor_copy(wt[:], ws[:])
                    wr[nm] = wt

                # ---- Q/K projections -> (hd parts, tok free) f32r ----
                qkt = {}
                for nm in ("q", "k"):
                    dest = qkvp.tile([P, T], F32R, name=f"{nm}T",
                                     tag=f"{nm}T")
                    for qc in range(NQC):
                        acc = psA.tile([P, TQ], F32, name="acc", tag="mmacc")
                        for kd in range(NKD):
                            nc.tensor.matmul(
                                acc[:],
                                wr[nm][:, kd * HD:(kd + 1) * HD],
                                xr[:, kd * T + qc * TQ: kd * T + (qc + 1) * TQ],
                                start=(kd == 0), stop=(kd == NKD - 1))
                        nc.vector.tensor_copy(
                            dest[:, qc * TQ:(qc + 1) * TQ], acc[:])
                    qkt[nm] = dest

                # ---- V projection -> (tok parts, hd free) f32r ----
                v_sb = qkvp.tile([P, NTT * HD], F32R, name="vT", tag="vT")
                for tt in range(NTT):
                    accv = psA.tile([P, HD], F32, name="accv", tag="mmacc")
                    for kd in range(NKD):
                        nc.tensor.matmul(
                            accv[:],
                            xr[:, kd * T + tt * P: kd * T + (tt + 1) * P],
                            wr["v"][:, kd * HD:(kd + 1) * HD],
                            start=(kd == 0), stop=(kd == NKD - 1))
                    nc.vector.tensor_copy(
                        v_sb[:, tt * HD:(tt + 1) * HD], accv[:])

                # ---- causal attention, scoresT layout ----
                out_b = outbp.tile([P, T], F32, name="out_b", tag="out_b")
                for qc in range(NQC):
                    nkt = 4 * (qc + 1)
                    acco = psO.tile([P, TQ], F32, name="acco", tag="acco")
                    part = scp.tile([P, TQ], F32, name="part", tag="part")
                    for kt in range(nkt):
                        accs = psS.tile([P, TQ], F32, name="accs", tag="accs")
                        nc.tensor.matmul(
                            accs[:],
                            qkt["k"][:, kt * P:(kt + 1) * P],
                            qkt["q"][:, qc * TQ:(qc + 1) * TQ],
                            start=True, stop=True)
                        e = eop.tile([P, TQ], F32R, name="e", tag="e")
                        nc.scalar.activation(e[:], accs[:], Exp)
                        j = kt - 4 * qc
                        if j >= 0:  # diagonal-crossing tile: zero invalid
                            nc.vector.tensor_mul(
                                e[:], e[:], cm_sb[:, j * TQ:(j + 1) * TQ])
                        if kt == 0:
                            nc.vector.tensor_copy(part[:], e[:])
                        else:
                            nc.vector.tensor_add(part[:], part[:], e[:])
                        nc.tensor.matmul(
                            acco[:],
                            v_sb[:, kt * HD:(kt + 1) * HD],
                            e[:],
                            start=(kt == 0), stop=(kt == nkt - 1))
                    # denominator: broadcast column-sum via exact fp32 matmul
                    denb = psA.tile([P, TQ], F32, name="denb", tag="mmacc")
                    nc.tensor.matmul(denb[:], ones128[:], part[:],
                                     start=True, stop=True)
                    recb = scp.tile([P, TQ], F32, name="recb", tag="recb")
                    nc.vector.reciprocal_approx_fast(recb[:], denb[:])
                    nc.vector.tensor_mul(
                        out_b[:, qc * TQ:(qc + 1) * TQ], acco[:], recb[:])

                # ---- ship normalized head-output to A2A input ----
                nc.sync.dma_start(a2a_in[2 * b], out_b[:, 0:TOK_SLICE])
                nc.sync.dma_start(a2a_in[2 * b + 1], out_b[:, TOK_SLICE:T])

            # ---- exchange: core c receives all heads for its token slice
            nc.gpsimd.collective_compute(
                "AllToAll", mybir.AluOpType.bypass, replica_groups=rg,
                ins=[a2a_in.opt()], outs=[a2a_out.opt()])

            # ---- multiplicative reduce over heads ----
            pr = tailp.tile([P, TOK_SLICE], F32, name="pr", tag="pr")
            prodr = tailp.tile([P, TOK_SLICE], F32R, name="prodr",
                               tag="prodr")
            for r in range(NC):
                hp = hpp.tile([P, TOK_SLICE], F32, name="hp", tag="hp")
                nc.gpsimd.dma_start(hp[:], a2a_out[r])
                if r == 0:
                    nc.vector.tensor_copy(pr[:], hp[:])
                elif r < NC - 1:
                    nc.vector.tensor_mul(pr[:], pr[:], hp[:])
                else:
                    nc.vector.tensor_mul(prodr[:], pr[:], hp[:])

            # ---- out_proj on the token slice ----
            for tt in range(TOK_SLICE // P):
                ysb = ysbp.tile([P, D], F32, name="ysb", tag="ysb")
                for nn in range(D // TQ):
                    accy = psA.tile([P, TQ], F32, name="accy", tag="mmacc")
                    nc.tensor.matmul(
                        accy[:],
                        prodr[:, tt * P:(tt + 1) * P],
                        wor[:, nn * TQ:(nn + 1) * TQ],
                        start=True, stop=True)
                    nc.vector.tensor_copy(
                        ysb[:, nn * TQ:(nn + 1) * TQ], accy[:])
                nc.sync.dma_start(y_d.ap()[tt * P:(tt + 1) * P, :], ysb[:])

    nc.compile()
    _PROGRAM = nc
    return nc


def make_in_maps(x, Wq, Wk, Wv, Wout, q_mask, k_mask, v_mask):
    x = np.ascontiguousarray(np.asarray(x, np.float32))
    xT = np.ascontiguousarray(x.transpose(0, 2, 1))        # (B, D, T)
    wo = np.ascontiguousarray(np.asarray(Wout, np.float32).T)  # (HD, D)

    cm = np.zeros((4, P, TQ), np.float32)
    for j in range(4):
        for i in range(P):
            cm[j, i, j * P + i:] = 1.0

    s = np.float32(1.0 / np.sqrt(HD))
    q_mask = np.asarray(q_mask, np.float32)
    k_mask = np.asarray(k_mask, np.float32)
    v_mask = np.asarray(v_mask, np.float32)
    Wq = np.asarray(Wq, np.float32)
    Wk = np.asarray(Wk, np.float32)
    Wv = np.asarray(Wv, np.float32)

    in_maps = []
    for c in range(NC):
        def pack(W, m, scale):
            # W rows for head c, rows scaled by mask (and 1/sqrt(HD) for q)
            out = np.empty((B, NKD, P, HD), np.float32)
            Wh = W[c * HD:(c + 1) * HD, :]                  # (HD, D)
            for b in range(B):
                Wp = (Wh * (m[b, c, 0, :, None] * scale)).T  # (D, HD)
                out[b] = Wp.reshape(NKD, P, HD)
            return out
        in_maps.append({
            "xT": xT,
            "wq": pack(Wq, q_mask, s),
            "wk": pack(Wk, k_mask, np.float32(1.0)),
            "wv": pack(Wv, v_mask, np.float32(1.0)),
            "wo": wo,
            "cm": cm,
        })
    return in_maps


def kernel(x, Wq, Wk, Wv, Wout, q_mask, k_mask, v_mask, mask=None):
    nc = build_program()
    in_maps = make_in_maps(x, Wq, Wk, Wv, Wout, q_mask, k_mask, v_mask)
    res = run_bass_kernel_spmd(nc, in_maps, core_ids=list(range(NC))).results
    y = np.stack([res[c]["y"] for c in range(NC)])  # (8, 1024, D)
    return y.reshape(B, T, D)
